# revision 6
# baseline (speedup 1.0000x reference)
"""Trainium2 Bass kernel for nn_CrossAttentionExpert (optimized v3).

Problem (hardcoded shapes): B=4, C=256, H=W=64 (N=4096), C8=32.
  cross_p2v = attn(q=wq_p@f_p, k=wk_v@f_v, v=wv_v@f_v)
  cross_v2p = attn(q=wq_v@f_v, k=wk_p@f_p, v=wv_p@f_p)
  out = BN(w_out @ concat([f_p, f_v, cross_p2v, cross_v2p]))  (training BN)

Sharding: 8 cores = (batch b, spatial half h).  Each core computes both
attention directions for its 2048 query positions (keys span all 4096
positions of its batch), plus BN with a [128,4] AllReduce of per-channel
sum/sumsq.

Design (see git history for the f32r baseline at 695us, v2 at 361us):
- All matmuls bf16 (1 col/cycle, FWL weight loads, less power throttle);
  inputs cast to bf16 host-side (halves DMA).
- Cross-term output conv folded into V host-side (wv' = w_out[:,cross]@wv)
  so AV directly produces y contributions; V-bias dropped entirely (it
  shifts y by a per-channel constant which training-mode BN cancels
  exactly); 1/rowsum applied to the folded 256-ch AV output.
- Scores S^T (keys on partitions feed AV with no transposes); the K=32
  contraction packed 4x via tile_position row tiling, with kt/qr
  replicated across partition groups for free by replicating the tiny
  conv weights 4x along stationary columns (the 4 concurrent matmuls
  share one moving-operand stream).
- Two-deep software pipeline over (dir, mtile): ACT exps tile t while the
  PE runs tile t-1's AV matmuls + spread-out conv "filler" work; PSUM =
  4-bank score group + 2-bank AV accumulator + 2 misc banks.
- Softmax denominator: contiguous bf16 pairwise adds (DVE) +
  gpsimd.partition_all_reduce (sum over key-partitions + broadcast) +
  reciprocal_approx_fast.  The av-scale of tile t-1 is emitted *before*
  tile t's denominator chain so the AV PSUM banks free without stalling
  the PE at mtile boundaries.
"""

import numpy as np
import ml_dtypes

import concourse.bass as bass
import concourse.bass_isa as bass_isa
import concourse.mybir as mybir
import concourse.tile as tile
from concourse import bacc, bass_utils

FP = mybir.dt.float32
BF = mybir.dt.bfloat16
P = 128
C = 256
C8 = 32
N = 4096          # keys per batch
M = 2048          # local query positions per core
NMT = 4           # m-tiles of 512
MT = 512
NCH = 32          # key chunks of 128 per m-tile
NGR = 8           # groups of 4 key chunks
NCORES = 8
BN_EPS = 1e-5
BN_COUNT = 4 * 4096

_ALU = mybir.AluOpType
_ACT = mybir.ActivationFunctionType

_PROGRAM = None


def _build_program():
    nc = bacc.Bacc("TRN2", target_bir_lowering=False, debug=False,
                   num_devices=NCORES)

    # ---- DRAM I/O ----
    # kv0 = f_v (rolled), kv1 = f_p (rolled), bf16
    kv = [nc.dram_tensor(f"kv{d}", [C, N], BF, kind="ExternalInput").ap()
          for d in range(2)]
    wq = [nc.dram_tensor(f"wq{d}", [C, P], BF, kind="ExternalInput").ap()
          for d in range(2)]
    wk = [nc.dram_tensor(f"wk{d}", [C, P], BF, kind="ExternalInput").ap()
          for d in range(2)]
    wv = [nc.dram_tensor(f"wv{d}", [C, C], BF, kind="ExternalInput").ap()
          for d in range(2)]
    wdir = nc.dram_tensor("wdir", [2 * C, C], BF, kind="ExternalInput").ap()
    qkbias = nc.dram_tensor("qkbias", [P, 4], FP, kind="ExternalInput").ap()
    gb = nc.dram_tensor("gb", [P, 4], FP, kind="ExternalInput").ap()
    yout = nc.dram_tensor("y", [C, M], FP, kind="ExternalOutput").ap()

    with tile.TileContext(nc) as tc:
        with (
            tc.tile_pool(name="consts", bufs=1) as consts,
            tc.tile_pool(name="big", bufs=1) as big,
            tc.tile_pool(name="kt", bufs=2) as p_kt,
            tc.tile_pool(name="qr", bufs=2) as p_qr,
            tc.tile_pool(name="vt", bufs=2) as p_vt,
            tc.tile_pool(name="stg", bufs=2) as p_stg,
            tc.tile_pool(name="row", bufs=2) as p_row,
            tc.tile_pool(name="small", bufs=4) as p_small,
            tc.tile_pool(name="ps4", bufs=1, space="PSUM") as ps4p,
            tc.tile_pool(name="psav", bufs=1, space="PSUM") as psavp,
            tc.tile_pool(name="psm", bufs=2, space="PSUM") as psm,
            tc.tile_pool(name="dram", bufs=1, space="DRAM") as dram,
        ):
            # ---- load constants first (small; conv matmuls need them
            # ---- before the big kv tensors finish) ----
            def load_w(ap, shape, name, dt=BF):
                t = consts.tile(shape, dt, name=name)
                nc.sync.dma_start(t[:], ap.rearrange("(o p) m -> p o m", p=P))
                return t

            wq_sb = [load_w(wq[d], [P, 2, P], f"wqsb{d}") for d in range(2)]
            wk_sb = [load_w(wk[d], [P, 2, P], f"wksb{d}") for d in range(2)]
            wv_sb = [load_w(wv[d], [P, 2, C], f"wvsb{d}") for d in range(2)]
            wdir_sb = load_w(wdir, [P, 4, C], "wdirsb")
            qkb_sb = consts.tile([P, 4], FP, name="qkbsb")
            nc.sync.dma_start(qkb_sb[:], qkbias[:])
            gb_sb = consts.tile([P, 4], FP, name="gbsb")
            nc.sync.dma_start(gb_sb[:], gb[:])

            kv_sb = []
            for d in range(2):
                t = big.tile([P, 2, N], BF, name=f"kvsb{d}")
                src = kv[d].rearrange("(o p) n -> p o n", p=P)
                for o in range(2):
                    for q in range(4):
                        sl = slice(q * 1024, (q + 1) * 1024)
                        nc.sync.dma_start(t[:, o, sl], src[:, o, sl])
                kv_sb.append(t)

            y_acc = [big.tile([P, M], FP, name=f"yacc{cc}") for cc in range(2)]
            stats_s = big.tile([P, 8], FP, name="stats_s")
            stats_q = big.tile([P, 8], FP, name="stats_q")
            scr = big.tile([P, MT], BF, name="scr")  # discard target

            # ---- per-dir persistent tiles (allocated up front; pools give
            # ---- each dir its own buffer) ----
            qr_t = [p_qr.tile([P, M], BF, tag="qr", name=f"qr{d}")
                    for d in range(2)]
            kt_t = [p_kt.tile([P, N], BF, tag="kt", name=f"kt{d}")
                    for d in range(2)]
            vt_t = [p_vt.tile([P, NCH, C], BF, tag="vt", name=f"vt{d}")
                    for d in range(2)]

            # ---- conv work units (each: a few matmuls + one DVE op) ----
            def unit_direct(oc, t):
                def emit():
                    msl = slice(t * MT, (t + 1) * MT)
                    ocs = slice(oc * P, (oc + 1) * P)
                    ps = psm.tile([P, MT], FP, tag="misc")
                    for j, (kvi, o) in enumerate(
                            ((1, 0), (1, 1), (0, 0), (0, 1))):
                        nc.tensor.matmul(ps, wdir_sb[:, j, ocs],
                                         kv_sb[kvi][:, o, slice(t * MT,
                                                                (t + 1) * MT)],
                                         start=(j == 0), stop=(j == 3))
                    nc.vector.tensor_copy(y_acc[oc][:, msl], ps)
                return emit

            def unit_qr(d, t):
                def emit():
                    qkv = kv_sb[1 - d]
                    msl = slice(t * MT, (t + 1) * MT)
                    ps = psm.tile([P, MT], FP, tag="misc")
                    for kc in range(2):
                        nc.tensor.matmul(ps, wq_sb[d][:, kc, :],
                                         qkv[:, kc, msl],
                                         start=(kc == 0), stop=(kc == 1))
                    nc.vector.tensor_scalar_add(
                        qr_t[d][:, msl], ps, qkb_sb[:, 2 * d:2 * d + 1])
                return emit

            def unit_kt(d, sub):
                def emit():
                    kkv = kv_sb[d]
                    nsl = slice(sub * MT, (sub + 1) * MT)
                    ps = psm.tile([P, MT], FP, tag="misc")
                    for kc in range(2):
                        nc.tensor.matmul(ps, wk_sb[d][:, kc, :],
                                         kkv[:, kc, nsl],
                                         start=(kc == 0), stop=(kc == 1))
                    nc.vector.tensor_scalar_add(
                        kt_t[d][:, nsl], ps, qkb_sb[:, 2 * d + 1:2 * d + 2])
                return emit

            def unit_vt(d, j2):
                def emit():
                    kkv = kv_sb[d]
                    ps = psm.tile([P, 2, C], FP, tag="misc")
                    for jj in range(2):
                        j = 2 * j2 + jj
                        for kc in range(2):
                            nc.tensor.matmul(
                                ps[:, jj, :],
                                kkv[:, kc, j * P:(j + 1) * P],
                                wv_sb[d][:, kc, :],
                                start=(kc == 0), stop=(kc == 1))
                    # split the PSUM->SBUF casts between DVE and ACT
                    dst = vt_t[d][:, 2 * j2:2 * j2 + 2, :]
                    if j2 % 2 == 0:
                        nc.vector.tensor_copy(dst, ps)
                    else:
                        nc.scalar.copy(dst, ps)
                return emit

            # filler schedule: tile index i = 4*d + t -> slot -> units
            fillers = {i: [[] for _ in range(NGR)] for i in range(8)}

            def spread(units, i):
                nslots = NGR
                for u, fn in enumerate(units):
                    fillers[i][(u * nslots) // len(units)].append(fn)

            spread([unit_direct(oc, t) for oc in range(2)
                    for t in range(NMT)] +
                   [unit_vt(0, j2) for j2 in range(16)], 0)
            spread([unit_qr(1, t) for t in range(NMT)] +
                   [unit_kt(1, sub) for sub in range(8)], 2)
            spread([unit_vt(1, j2) for j2 in range(8)], 3)
            spread([unit_vt(1, j2) for j2 in range(8, 16)], 4)

            # prologue: dir0 q/k convs only
            for t in range(NMT):
                unit_qr(0, t)()
            for sub in range(8):
                unit_kt(0, sub)()

            # ---- software pipeline over (dir, mtile) ----
            tiles = [(d, t) for d in range(2) for t in range(NMT)]
            prev = None  # (d, t, stg, av, rinv, msl)

            def emit_av_group(pv, g):
                d_, t_, stg_, av_, _, _ = pv
                for i in range(4):
                    ch = 4 * g + i
                    for cc in range(2):
                        nc.tensor.matmul(
                            av_[:, cc, :],
                            vt_t[d_][:, ch, cc * P:(cc + 1) * P],
                            stg_[:, ch, :],
                            start=(g == 0 and i == 0),
                            stop=(g == NGR - 1 and i == 3),
                            skip_group_check=True)

            def finish_prev(pv):
                """Scale prev tile's AV output by 1/rowsum into y_acc and,
                for dir1 tiles, fold BN partial stats."""
                d_, t_, _, av_, rinv_, msl_ = pv
                for cc in range(2):
                    tmp = p_small.tile([P, MT], FP, tag="avtmp")
                    nc.vector.tensor_mul(tmp[:], av_[:, cc, :], rinv_[:])
                    nc.vector.tensor_add(y_acc[cc][:, msl_],
                                         y_acc[cc][:, msl_], tmp[:])
                if d_ == 1:
                    for cc in range(2):
                        col = slice(cc * 4 + t_, cc * 4 + t_ + 1)
                        nc.scalar.activation(
                            scr[:], y_acc[cc][:, msl_], _ACT.Square,
                            accum_out=stats_q[:, col])
                        nc.vector.reduce_sum(stats_s[:, col],
                                             y_acc[cc][:, msl_],
                                             axis=mybir.AxisListType.X)

            for (d, t) in tiles:
                qr, kt, vt = qr_t[d], kt_t[d], vt_t[d]
                i = 4 * d + t
                msl = slice(t * MT, (t + 1) * MT)
                stg = p_stg.tile([P, NCH, MT], BF, tag="stg")
                av = psavp.tile([P, 2, MT], FP, tag="av")
                racc = p_row.tile([P, MT], FP, tag="racc")
                for g in range(NGR):
                    ps = ps4p.tile([P, 4, MT], FP, tag="ps4")
                    for q in range(4):
                        ch = 4 * g + q
                        nc.tensor.matmul(
                            ps[:, q, :],
                            kt[32 * q:32 * (q + 1), ch * P:(ch + 1) * P],
                            qr[32 * q:32 * (q + 1), msl],
                            start=True, stop=True,
                            tile_position=(32 * q, 0))
                    if prev is not None:
                        emit_av_group(prev, g)
                    for fn in fillers[i][g]:
                        fn()
                    nc.scalar.activation(stg[:, 4 * g:4 * g + 4, :], ps[:],
                                         _ACT.Exp)
                    if g == NGR - 1 and prev is not None:
                        # free prev's AV banks before this tile's denominator
                        # chain so the next tile's AV matmuls aren't stalled
                        finish_prev(prev)
                    # rowsum partials: first pairwise level on DVE (bf16
                    # double-rate), second level on GpSimd, fp32 accumulate
                    # back on DVE
                    t1 = p_small.tile([P, 2, MT], BF, tag="t1")
                    nc.vector.tensor_add(t1[:], stg[:, 4 * g:4 * g + 2, :],
                                         stg[:, 4 * g + 2:4 * g + 4, :])
                    t2 = p_small.tile([P, MT], BF, tag="t2")
                    nc.gpsimd.tensor_add(t2[:], t1[:, 0, :], t1[:, 1, :])
                    if g == 0:
                        nc.vector.tensor_copy(racc[:], t2[:])
                    else:
                        nc.vector.tensor_add(racc[:], racc[:], t2[:])
                rbc = p_row.tile([P, MT], FP, tag="rbc")
                nc.gpsimd.partition_all_reduce(rbc[:], racc[:], P,
                                               bass_isa.ReduceOp.add)
                rinv = p_row.tile([P, MT], FP, tag="rinv")
                nc.vector.reciprocal_approx_fast(out=rinv[:], in_=rbc[:])
                prev = (d, t, stg, av, rinv, msl)

            # drain: last tile's AV + scale + stats
            for g in range(NGR):
                emit_av_group(prev, g)
            finish_prev(prev)

            # ---- BN: pack stats, AllReduce, normalize ----
            stats = p_small.tile([P, 4], FP, tag="stats")
            for cc in range(2):
                nc.vector.reduce_sum(stats[:, cc:cc + 1],
                                     stats_s[:, 4 * cc:4 * cc + 4],
                                     axis=mybir.AxisListType.X)
                nc.vector.reduce_sum(stats[:, 2 + cc:3 + cc],
                                     stats_q[:, 4 * cc:4 * cc + 4],
                                     axis=mybir.AxisListType.X)
            cc_in = dram.tile([P, 4], FP)
            cc_out = dram.tile([P, 4], FP)
            nc.sync.dma_start(cc_in[:], stats[:])
            nc.gpsimd.collective_compute(
                "AllReduce", _ALU.add,
                replica_groups=[list(range(NCORES))],
                ins=[cc_in.opt()], outs=[cc_out.opt()])
            ar = p_small.tile([P, 4], FP, tag="ar")
            nc.sync.dma_start(ar[:], cc_out[:])

            inv_n = 1.0 / BN_COUNT
            yo = yout.rearrange("(o p) m -> p o m", p=P)
            for cc in range(2):
                mean = p_small.tile([P, 1], FP, tag="bn")
                ex2 = p_small.tile([P, 1], FP, tag="bn")
                var = p_small.tile([P, 1], FP, tag="bn")
                nc.vector.tensor_scalar_mul(mean[:], ar[:, cc:cc + 1], inv_n)
                nc.vector.tensor_scalar_mul(ex2[:], ar[:, 2 + cc:3 + cc],
                                            inv_n)
                nc.vector.tensor_tensor(var[:], mean[:], mean[:], _ALU.mult)
                nc.vector.tensor_sub(var[:], ex2[:], var[:])
                sd = p_small.tile([P, 1], FP, tag="bn")
                nc.vector.tensor_scalar_add(var[:], var[:], BN_EPS)
                nc.scalar.activation(sd[:], var[:], _ACT.Sqrt)
                rstd = p_small.tile([P, 1], FP, tag="bn")
                nc.vector.reciprocal(rstd[:], sd[:])
                scale = p_small.tile([P, 1], FP, tag="bn")
                nc.vector.tensor_tensor(scale[:], gb_sb[:, cc:cc + 1],
                                        rstd[:], _ALU.mult)
                shift = p_small.tile([P, 1], FP, tag="bn")
                nc.vector.tensor_tensor(shift[:], mean[:], scale[:],
                                        _ALU.mult)
                nc.vector.tensor_sub(shift[:], gb_sb[:, 2 + cc:3 + cc],
                                     shift[:])
                for q in range(2):
                    qsl = slice(q * 1024, (q + 1) * 1024)
                    nc.vector.tensor_scalar(
                        out=y_acc[cc][:, qsl], in0=y_acc[cc][:, qsl],
                        scalar1=scale[:], scalar2=shift[:],
                        op0=_ALU.mult, op1=_ALU.add)
                    nc.sync.dma_start(yo[:, cc, qsl], y_acc[cc][:, qsl])

    nc.compile()
    return nc


def _get_program():
    global _PROGRAM
    if _PROGRAM is None:
        _PROGRAM = _build_program()
    return _PROGRAM


def _bf(x):
    return np.ascontiguousarray(np.asarray(x, np.float32)).astype(
        ml_dtypes.bfloat16)


def _make_in_maps(inputs):
    f_p = np.ascontiguousarray(
        np.asarray(inputs["f_p"], np.float32).reshape(4, C, N))
    f_v = np.ascontiguousarray(
        np.asarray(inputs["f_v"], np.float32).reshape(4, C, N))

    w_out = np.asarray(inputs["w_out"], np.float32)

    def rep4(w):  # [32, 256] -> [256, 128] (4 col-copies of w^T)
        return np.tile(np.asarray(w, np.float32).T, (1, 4))

    def fused_v(dcol, wv_):  # (w_out[:, dcol] @ wv)^T [256, 256]
        blk = w_out[:, dcol * C:(dcol + 1) * C]
        return (blk @ np.asarray(wv_, np.float32)).T

    def tile4(b):  # [32] -> [128]
        return np.tile(np.asarray(b, np.float32), 4)

    shared = {
        # dir0 (p2v): q from f_p, k/v from f_v; dir1 (v2p): reversed
        "wq0": _bf(rep4(inputs["wq_p"])), "wk0": _bf(rep4(inputs["wk_v"])),
        "wv0": _bf(fused_v(2, inputs["wv_v"])),
        "wq1": _bf(rep4(inputs["wq_v"])), "wk1": _bf(rep4(inputs["wk_p"])),
        "wv1": _bf(fused_v(3, inputs["wv_p"])),
        "wdir": _bf(w_out[:, :2 * C].T),
        "qkbias": np.ascontiguousarray(np.stack(
            [tile4(inputs["bq_p"]), tile4(inputs["bk_v"]),
             tile4(inputs["bq_v"]), tile4(inputs["bk_p"])], axis=1)),
        "gb": np.ascontiguousarray(np.stack(
            [np.asarray(inputs["gamma"], np.float32)[:P],
             np.asarray(inputs["gamma"], np.float32)[P:],
             np.asarray(inputs["beta"], np.float32)[:P],
             np.asarray(inputs["beta"], np.float32)[P:]], axis=1)),
    }
    in_maps = []
    for core in range(NCORES):
        b, h = divmod(core, 2)
        # roll so this core's query half sits at columns [0, 2048); K/V use
        # the full (permuted) range — softmax/AV are order-invariant in keys.
        kv1 = _bf(np.roll(f_p[b], -h * M, axis=1))
        kv0 = _bf(np.roll(f_v[b], -h * M, axis=1))
        in_maps.append({"kv0": kv0, "kv1": kv1, **shared})
    return in_maps


def _assemble(results):
    out = np.empty((4, C, N), np.float32)
    for core in range(NCORES):
        b, h = divmod(core, 2)
        out[b][:, h * M:(h + 1) * M] = results[core]["y"]
    return out.reshape(4, C, 64, 64)


def _run(inputs, **kwargs):
    nc = _get_program()
    in_maps = _make_in_maps(inputs)
    res = bass_utils.run_bass_kernel_spmd(
        nc, in_maps, core_ids=list(range(NCORES)), **kwargs)
    return _assemble(res.results), res


def kernel(**inputs):
    out, _ = _run(inputs)
    return out


# revision 7
# speedup vs baseline: 1.2728x; 1.2728x over previous
"""Trainium2 Bass kernel for nn_CrossAttentionExpert (optimized v3).

Problem (hardcoded shapes): B=4, C=256, H=W=64 (N=4096), C8=32.
  cross_p2v = attn(q=wq_p@f_p, k=wk_v@f_v, v=wv_v@f_v)
  cross_v2p = attn(q=wq_v@f_v, k=wk_p@f_p, v=wv_p@f_p)
  out = BN(w_out @ concat([f_p, f_v, cross_p2v, cross_v2p]))  (training BN)

Sharding: 8 cores = (batch b, spatial half h).  Each core computes both
attention directions for its 2048 query positions (keys span all 4096
positions of its batch), plus BN with a [128,4] AllReduce of per-channel
sum/sumsq.

Design (see git history for the f32r baseline at 695us, v2 at 361us):
- All matmuls bf16 (1 col/cycle, FWL weight loads, less power throttle);
  inputs cast to bf16 host-side (halves DMA).
- Cross-term output conv folded into V host-side (wv' = w_out[:,cross]@wv)
  so AV directly produces y contributions; V-bias dropped entirely (it
  shifts y by a per-channel constant which training-mode BN cancels
  exactly); 1/rowsum applied to the folded 256-ch AV output.
- Scores S^T (keys on partitions feed AV with no transposes); the K=32
  contraction packed 4x via tile_position row tiling, with kt/qr
  replicated across partition groups for free by replicating the tiny
  conv weights 4x along stationary columns (the 4 concurrent matmuls
  share one moving-operand stream).
- Two-deep software pipeline over (dir, mtile): ACT exps tile t while the
  PE runs tile t-1's AV matmuls + spread-out conv "filler" work; PSUM =
  4-bank score group + 2-bank AV accumulator + 2 misc banks.
- Softmax denominator: contiguous bf16 pairwise adds (DVE) +
  gpsimd.partition_all_reduce (sum over key-partitions + broadcast) +
  reciprocal_approx_fast.  The av-scale of tile t-1 is emitted *before*
  tile t's denominator chain so the AV PSUM banks free without stalling
  the PE at mtile boundaries.
"""

import numpy as np
import ml_dtypes

import concourse.bass as bass
import concourse.bass_isa as bass_isa
import concourse.mybir as mybir
import concourse.tile as tile
from concourse import bacc, bass_utils

FP = mybir.dt.float32
BF = mybir.dt.bfloat16
P = 128
C = 256
C8 = 32
N = 4096          # keys per batch
M = 2048          # local query positions per core
NMT = 4           # m-tiles of 512
MT = 512
NCH = 32          # key chunks of 128 per m-tile
NGR = 8           # groups of 4 key chunks
NCORES = 8
BN_EPS = 1e-5
BN_COUNT = 4 * 4096

_ALU = mybir.AluOpType
_ACT = mybir.ActivationFunctionType

_PROGRAM = None


def _build_program():
    nc = bacc.Bacc("TRN2", target_bir_lowering=False, debug=False,
                   num_devices=NCORES)

    # ---- DRAM I/O ----
    # kv0 = f_v (rolled), kv1 = f_p (rolled), bf16
    kv = [nc.dram_tensor(f"kv{d}", [C, N], BF, kind="ExternalInput").ap()
          for d in range(2)]
    wq = [nc.dram_tensor(f"wq{d}", [C, P], BF, kind="ExternalInput").ap()
          for d in range(2)]
    wk = [nc.dram_tensor(f"wk{d}", [C, P], BF, kind="ExternalInput").ap()
          for d in range(2)]
    wv = [nc.dram_tensor(f"wv{d}", [C, C], BF, kind="ExternalInput").ap()
          for d in range(2)]
    wdir = nc.dram_tensor("wdir", [2 * C, C], BF, kind="ExternalInput").ap()
    qkbias = nc.dram_tensor("qkbias", [P, 4], FP, kind="ExternalInput").ap()
    gb = nc.dram_tensor("gb", [P, 4], FP, kind="ExternalInput").ap()
    yout = nc.dram_tensor("y", [C, M], FP, kind="ExternalOutput").ap()

    with tile.TileContext(nc) as tc:
        with (
            tc.tile_pool(name="consts", bufs=1) as consts,
            tc.tile_pool(name="big", bufs=1) as big,
            tc.tile_pool(name="kt", bufs=2) as p_kt,
            tc.tile_pool(name="qr", bufs=2) as p_qr,
            tc.tile_pool(name="vt", bufs=2) as p_vt,
            tc.tile_pool(name="stg", bufs=2) as p_stg,
            tc.tile_pool(name="row", bufs=2) as p_row,
            tc.tile_pool(name="small", bufs=4) as p_small,
            tc.tile_pool(name="ps4", bufs=1, space="PSUM") as ps4p,
            tc.tile_pool(name="psav", bufs=1, space="PSUM") as psavp,
            tc.tile_pool(name="psm", bufs=2, space="PSUM") as psm,
            tc.tile_pool(name="dram", bufs=1, space="DRAM") as dram,
        ):
            # ---- load constants first (small; conv matmuls need them
            # ---- before the big kv tensors finish) ----
            def load_w(ap, shape, name, dt=BF):
                t = consts.tile(shape, dt, name=name)
                nc.sync.dma_start(t[:], ap.rearrange("(o p) m -> p o m", p=P))
                return t

            wq_sb = [load_w(wq[d], [P, 2, P], f"wqsb{d}") for d in range(2)]
            wk_sb = [load_w(wk[d], [P, 2, P], f"wksb{d}") for d in range(2)]
            wv_sb = [load_w(wv[d], [P, 2, C], f"wvsb{d}") for d in range(2)]
            wdir_sb = load_w(wdir, [P, 4, C], "wdirsb")
            qkb_sb = consts.tile([P, 4], FP, name="qkbsb")
            nc.sync.dma_start(qkb_sb[:], qkbias[:])
            gb_sb = consts.tile([P, 4], FP, name="gbsb")
            nc.sync.dma_start(gb_sb[:], gb[:])

            kv_sb = []
            for d in range(2):
                t = big.tile([P, 2, N], BF, name=f"kvsb{d}")
                src = kv[d].rearrange("(o p) n -> p o n", p=P)
                for o in range(2):
                    for q in range(4):
                        sl = slice(q * 1024, (q + 1) * 1024)
                        nc.sync.dma_start(t[:, o, sl], src[:, o, sl])
                kv_sb.append(t)

            y_acc = [big.tile([P, M], FP, name=f"yacc{cc}") for cc in range(2)]
            stats_s = big.tile([P, 8], FP, name="stats_s")
            stats_q = big.tile([P, 8], FP, name="stats_q")
            scr = big.tile([P, MT], BF, name="scr")  # discard target

            # ---- per-dir persistent tiles (allocated up front; pools give
            # ---- each dir its own buffer) ----
            qr_t = [p_qr.tile([P, M], BF, tag="qr", name=f"qr{d}")
                    for d in range(2)]
            kt_t = [p_kt.tile([P, N], BF, tag="kt", name=f"kt{d}")
                    for d in range(2)]
            vt_t = [p_vt.tile([P, NCH, C], BF, tag="vt", name=f"vt{d}")
                    for d in range(2)]

            # ---- conv work units (each: a few matmuls + one DVE op) ----
            def unit_direct(oc, t):
                def emit():
                    msl = slice(t * MT, (t + 1) * MT)
                    ocs = slice(oc * P, (oc + 1) * P)
                    ps = psm.tile([P, MT], FP, tag="misc")
                    for j, (kvi, o) in enumerate(
                            ((1, 0), (1, 1), (0, 0), (0, 1))):
                        nc.tensor.matmul(ps, wdir_sb[:, j, ocs],
                                         kv_sb[kvi][:, o, slice(t * MT,
                                                                (t + 1) * MT)],
                                         start=(j == 0), stop=(j == 3))
                    nc.vector.tensor_copy(y_acc[oc][:, msl], ps)
                return emit

            def unit_qr(d, t):
                def emit():
                    qkv = kv_sb[1 - d]
                    msl = slice(t * MT, (t + 1) * MT)
                    ps = psm.tile([P, MT], FP, tag="misc")
                    for kc in range(2):
                        nc.tensor.matmul(ps, wq_sb[d][:, kc, :],
                                         qkv[:, kc, msl],
                                         start=(kc == 0), stop=(kc == 1))
                    nc.vector.tensor_scalar_add(
                        qr_t[d][:, msl], ps, qkb_sb[:, 2 * d:2 * d + 1])
                return emit

            def unit_kt(d, sub):
                def emit():
                    kkv = kv_sb[d]
                    nsl = slice(sub * MT, (sub + 1) * MT)
                    ps = psm.tile([P, MT], FP, tag="misc")
                    for kc in range(2):
                        nc.tensor.matmul(ps, wk_sb[d][:, kc, :],
                                         kkv[:, kc, nsl],
                                         start=(kc == 0), stop=(kc == 1))
                    nc.vector.tensor_scalar_add(
                        kt_t[d][:, nsl], ps, qkb_sb[:, 2 * d + 1:2 * d + 2])
                return emit

            def unit_vt(d, j2):
                def emit():
                    kkv = kv_sb[d]
                    ps = psm.tile([P, 2, C], FP, tag="misc")
                    for jj in range(2):
                        j = 2 * j2 + jj
                        for kc in range(2):
                            nc.tensor.matmul(
                                ps[:, jj, :],
                                kkv[:, kc, j * P:(j + 1) * P],
                                wv_sb[d][:, kc, :],
                                start=(kc == 0), stop=(kc == 1))
                    # split the PSUM->SBUF casts between DVE and ACT
                    dst = vt_t[d][:, 2 * j2:2 * j2 + 2, :]
                    if j2 % 2 == 0:
                        nc.vector.tensor_copy(dst, ps)
                    else:
                        nc.scalar.copy(dst, ps)
                return emit

            # filler schedule: tile index i = 4*d + t -> slot -> units
            fillers = {i: [[] for _ in range(NGR)] for i in range(8)}

            def spread(units, i):
                nslots = NGR
                for u, fn in enumerate(units):
                    fillers[i][(u * nslots) // len(units)].append(fn)

            spread([unit_direct(oc, t) for oc in range(2)
                    for t in range(NMT)] +
                   [unit_vt(0, j2) for j2 in range(16)], 0)
            spread([unit_qr(1, t) for t in range(NMT)] +
                   [unit_kt(1, sub) for sub in range(8)], 2)
            spread([unit_vt(1, j2) for j2 in range(8)], 3)
            spread([unit_vt(1, j2) for j2 in range(8, 16)], 4)

            # prologue: dir0 q/k convs only
            for t in range(NMT):
                unit_qr(0, t)()
            for sub in range(8):
                unit_kt(0, sub)()

            # ---- software pipeline over (dir, mtile) ----
            tiles = [(d, t) for d in range(2) for t in range(NMT)]
            prev = None  # (d, t, stg, av, rinv, msl)

            def emit_av_group(pv, g):
                d_, t_, stg_, av_, _, _ = pv
                for i in range(4):
                    ch = 4 * g + i
                    for cc in range(2):
                        nc.tensor.matmul(
                            av_[:, cc, :],
                            vt_t[d_][:, ch, cc * P:(cc + 1) * P],
                            stg_[:, ch, :],
                            start=(g == 0 and i == 0),
                            stop=(g == NGR - 1 and i == 3),
                            skip_group_check=True)

            def finish_prev(pv):
                """Scale prev tile's AV output by 1/rowsum into y_acc and,
                for dir1 tiles, fold BN partial stats."""
                d_, t_, _, av_, rinv_, msl_ = pv
                for cc in range(2):
                    tmp = p_small.tile([P, MT], FP, tag="avtmp")
                    nc.vector.tensor_mul(tmp[:], av_[:, cc, :], rinv_[:])
                    nc.vector.tensor_add(y_acc[cc][:, msl_],
                                         y_acc[cc][:, msl_], tmp[:])
                if d_ == 1:
                    for cc in range(2):
                        col = slice(cc * 4 + t_, cc * 4 + t_ + 1)
                        nc.scalar.activation(
                            scr[:], y_acc[cc][:, msl_], _ACT.Square,
                            accum_out=stats_q[:, col])
                        nc.vector.reduce_sum(stats_s[:, col],
                                             y_acc[cc][:, msl_],
                                             axis=mybir.AxisListType.X)

            for (d, t) in tiles:
                qr, kt, vt = qr_t[d], kt_t[d], vt_t[d]
                i = 4 * d + t
                msl = slice(t * MT, (t + 1) * MT)
                stg = p_stg.tile([P, NCH, MT], BF, tag="stg")
                av = psavp.tile([P, 2, MT], FP, tag="av")
                racc = p_row.tile([P, MT], FP, tag="racc")
                for g in range(NGR):
                    ps = ps4p.tile([P, 4, MT], FP, tag="ps4")
                    for q in range(4):
                        ch = 4 * g + q
                        nc.tensor.matmul(
                            ps[:, q, :],
                            kt[32 * q:32 * (q + 1), ch * P:(ch + 1) * P],
                            qr[32 * q:32 * (q + 1), msl],
                            start=True, stop=True,
                            tile_position=(32 * q, 0))
                    if prev is not None:
                        emit_av_group(prev, g)
                    for fn in fillers[i][g]:
                        fn()
                    nc.scalar.activation(stg[:, 4 * g:4 * g + 4, :], ps[:],
                                         _ACT.Exp)
                    if g == NGR - 1 and prev is not None:
                        # free prev's AV banks before this tile's denominator
                        # chain so the next tile's AV matmuls aren't stalled
                        finish_prev(prev)
                    # rowsum partials: first pairwise level on DVE (bf16
                    # double-rate), second level on GpSimd, fp32 accumulate
                    # back on DVE
                    t1 = p_small.tile([P, 2, MT], BF, tag="t1")
                    nc.vector.tensor_add(t1[:], stg[:, 4 * g:4 * g + 2, :],
                                         stg[:, 4 * g + 2:4 * g + 4, :])
                    t2 = p_small.tile([P, MT], BF, tag="t2")
                    nc.vector.tensor_add(t2[:], t1[:, 0, :], t1[:, 1, :])
                    if g == 0:
                        nc.vector.tensor_copy(racc[:], t2[:])
                    else:
                        nc.vector.tensor_add(racc[:], racc[:], t2[:])
                rbc = p_row.tile([P, MT], FP, tag="rbc")
                nc.gpsimd.partition_all_reduce(rbc[:], racc[:], P,
                                               bass_isa.ReduceOp.add)
                rinv = p_row.tile([P, MT], FP, tag="rinv")
                nc.vector.reciprocal_approx_fast(out=rinv[:], in_=rbc[:])
                prev = (d, t, stg, av, rinv, msl)

            # drain: last tile's AV + scale + stats
            for g in range(NGR):
                emit_av_group(prev, g)
            finish_prev(prev)

            # ---- BN: pack stats, AllReduce, normalize ----
            stats = p_small.tile([P, 4], FP, tag="stats")
            for cc in range(2):
                nc.vector.reduce_sum(stats[:, cc:cc + 1],
                                     stats_s[:, 4 * cc:4 * cc + 4],
                                     axis=mybir.AxisListType.X)
                nc.vector.reduce_sum(stats[:, 2 + cc:3 + cc],
                                     stats_q[:, 4 * cc:4 * cc + 4],
                                     axis=mybir.AxisListType.X)
            cc_in = dram.tile([P, 4], FP)
            cc_out = dram.tile([P, 4], FP)
            nc.sync.dma_start(cc_in[:], stats[:])
            nc.gpsimd.collective_compute(
                "AllReduce", _ALU.add,
                replica_groups=[list(range(NCORES))],
                ins=[cc_in.opt()], outs=[cc_out.opt()])
            ar = p_small.tile([P, 4], FP, tag="ar")
            nc.sync.dma_start(ar[:], cc_out[:])

            inv_n = 1.0 / BN_COUNT
            yo = yout.rearrange("(o p) m -> p o m", p=P)
            for cc in range(2):
                mean = p_small.tile([P, 1], FP, tag="bn")
                ex2 = p_small.tile([P, 1], FP, tag="bn")
                var = p_small.tile([P, 1], FP, tag="bn")
                nc.vector.tensor_scalar_mul(mean[:], ar[:, cc:cc + 1], inv_n)
                nc.vector.tensor_scalar_mul(ex2[:], ar[:, 2 + cc:3 + cc],
                                            inv_n)
                nc.vector.tensor_tensor(var[:], mean[:], mean[:], _ALU.mult)
                nc.vector.tensor_sub(var[:], ex2[:], var[:])
                sd = p_small.tile([P, 1], FP, tag="bn")
                nc.vector.tensor_scalar_add(var[:], var[:], BN_EPS)
                nc.scalar.activation(sd[:], var[:], _ACT.Sqrt)
                rstd = p_small.tile([P, 1], FP, tag="bn")
                nc.vector.reciprocal(rstd[:], sd[:])
                scale = p_small.tile([P, 1], FP, tag="bn")
                nc.vector.tensor_tensor(scale[:], gb_sb[:, cc:cc + 1],
                                        rstd[:], _ALU.mult)
                shift = p_small.tile([P, 1], FP, tag="bn")
                nc.vector.tensor_tensor(shift[:], mean[:], scale[:],
                                        _ALU.mult)
                nc.vector.tensor_sub(shift[:], gb_sb[:, 2 + cc:3 + cc],
                                     shift[:])
                for q in range(2):
                    qsl = slice(q * 1024, (q + 1) * 1024)
                    nc.vector.tensor_scalar(
                        out=y_acc[cc][:, qsl], in0=y_acc[cc][:, qsl],
                        scalar1=scale[:], scalar2=shift[:],
                        op0=_ALU.mult, op1=_ALU.add)
                    nc.sync.dma_start(yo[:, cc, qsl], y_acc[cc][:, qsl])

    nc.compile()
    return nc


def _get_program():
    global _PROGRAM
    if _PROGRAM is None:
        _PROGRAM = _build_program()
    return _PROGRAM


def _bf(x):
    return np.ascontiguousarray(np.asarray(x, np.float32)).astype(
        ml_dtypes.bfloat16)


def _make_in_maps(inputs):
    f_p = np.ascontiguousarray(
        np.asarray(inputs["f_p"], np.float32).reshape(4, C, N))
    f_v = np.ascontiguousarray(
        np.asarray(inputs["f_v"], np.float32).reshape(4, C, N))

    w_out = np.asarray(inputs["w_out"], np.float32)

    def rep4(w):  # [32, 256] -> [256, 128] (4 col-copies of w^T)
        return np.tile(np.asarray(w, np.float32).T, (1, 4))

    def fused_v(dcol, wv_):  # (w_out[:, dcol] @ wv)^T [256, 256]
        blk = w_out[:, dcol * C:(dcol + 1) * C]
        return (blk @ np.asarray(wv_, np.float32)).T

    def tile4(b):  # [32] -> [128]
        return np.tile(np.asarray(b, np.float32), 4)

    shared = {
        # dir0 (p2v): q from f_p, k/v from f_v; dir1 (v2p): reversed
        "wq0": _bf(rep4(inputs["wq_p"])), "wk0": _bf(rep4(inputs["wk_v"])),
        "wv0": _bf(fused_v(2, inputs["wv_v"])),
        "wq1": _bf(rep4(inputs["wq_v"])), "wk1": _bf(rep4(inputs["wk_p"])),
        "wv1": _bf(fused_v(3, inputs["wv_p"])),
        "wdir": _bf(w_out[:, :2 * C].T),
        "qkbias": np.ascontiguousarray(np.stack(
            [tile4(inputs["bq_p"]), tile4(inputs["bk_v"]),
             tile4(inputs["bq_v"]), tile4(inputs["bk_p"])], axis=1)),
        "gb": np.ascontiguousarray(np.stack(
            [np.asarray(inputs["gamma"], np.float32)[:P],
             np.asarray(inputs["gamma"], np.float32)[P:],
             np.asarray(inputs["beta"], np.float32)[:P],
             np.asarray(inputs["beta"], np.float32)[P:]], axis=1)),
    }
    in_maps = []
    for core in range(NCORES):
        b, h = divmod(core, 2)
        # roll so this core's query half sits at columns [0, 2048); K/V use
        # the full (permuted) range — softmax/AV are order-invariant in keys.
        kv1 = _bf(np.roll(f_p[b], -h * M, axis=1))
        kv0 = _bf(np.roll(f_v[b], -h * M, axis=1))
        in_maps.append({"kv0": kv0, "kv1": kv1, **shared})
    return in_maps


def _assemble(results):
    out = np.empty((4, C, N), np.float32)
    for core in range(NCORES):
        b, h = divmod(core, 2)
        out[b][:, h * M:(h + 1) * M] = results[core]["y"]
    return out.reshape(4, C, 64, 64)


def _run(inputs, **kwargs):
    nc = _get_program()
    in_maps = _make_in_maps(inputs)
    res = bass_utils.run_bass_kernel_spmd(
        nc, in_maps, core_ids=list(range(NCORES)), **kwargs)
    return _assemble(res.results), res


def kernel(**inputs):
    out, _ = _run(inputs)
    return out
